# revision 34
# baseline (speedup 1.0000x reference)
"""GCN (3-layer + global mean pool + FC/sigmoid) on 8 Trainium2 NeuronCores, v2.

Aggregate-first node-sharded design. One canonical per-core edge order —
sorted by (dst tile, src parity, src) — is shared by all three layers: the
same fp8 one-hot scatter matrices drive layer 1/2/3 chunk matmuls, and
self-loops use per-tile bf16 diagonal matmuls from SBUF slabs.

Layer 1 needs no device gather at all: the host pre-gathers x[src] into the
canonical stream (xg, fp8). Layers 2/3 gather h1/h2 rows from fp8
"pair" tables ([N/2, 2F], one pair per 256B+ row, int16 pair indices) that
are exchanged with a single AllGather each. Tables, one-hot norms, and the
gathered message blocks are all fp8 (validated ~1e-3 max-rel); slabs,
weights, and the pooling path stay bf16/f32.
"""
import sys
import os

for _p in ("/opt/trn_rl_repo", "/root/.axon_site/_ro/trn_rl_repo"):
    if os.path.isdir(_p) and _p not in sys.path:
        sys.path.append(_p)

import numpy as np
import ml_dtypes

bf16 = ml_dtypes.bfloat16
f8 = ml_dtypes.float8_e4m3

N = 50000
E = 150000
G = 256
NC = 8
SH = N // NC             # 6250 nodes per core
TPC = (SH + 127) // 128  # 49 tiles per core
H1, H2, H3 = 128, 256, 512
NSEG = int(os.environ.get('KNSEG', '8'))  # gathers per layer (<= 8 DMASW lanes)

TRACE = False
LAST_EXEC_NS = None
_CACHE = {}
LOOKAHEAD = int(os.environ.get('KLOOKAHEAD', '14'))
GBUFS = int(os.environ.get('KGBUFS', '16'))
AGCHUNK = int(os.environ.get('KAGCHUNK', '1'))


def _prep(x, edge_index, edge_weight, batch):
    """Host-side graph preprocessing -> per-core input arrays + structure."""
    x = np.asarray(x, np.float32)
    ei = np.asarray(edge_index)
    ew = np.asarray(edge_weight, np.float32)
    batch = np.asarray(batch).astype(np.int64)

    src = ei[0].astype(np.int64)
    dst = ei[1].astype(np.int64)
    deg = np.bincount(dst, weights=ew, minlength=N).astype(np.float32) + 1.0
    dinv = (1.0 / np.sqrt(deg)).astype(np.float32)
    norm = (dinv[src] * ew * dinv[dst]).astype(np.float32)
    norm_self = (dinv * dinv).astype(np.float32)

    core = dst // SH
    tile_l = (dst % SH) // 128
    par = src % 2
    key = (core * TPC + tile_l) * 2 + par
    order = np.lexsort((src, key))
    src_s, dst_s, norm_s, key_s = src[order], dst[order], norm[order], key[order]

    cnt = np.bincount(key, minlength=NC * TPC * 2).reshape(NC, TPC, 2)
    ch_uni = np.ceil(cnt / 128).astype(np.int64).max(axis=0)   # [TPC, 2]
    CH = int(ch_uni.sum())
    # global block index of (tile, parity, k): blocks laid out tile-major
    blk_base = np.zeros((TPC, 2), np.int64)
    running = 0
    for t in range(TPC):
        for p in range(2):
            blk_base[t, p] = running
            running += ch_uni[t, p]

    block_start = np.zeros(NC * TPC * 2 + 1, np.int64)
    block_start[1:] = np.cumsum(cnt.reshape(-1))

    cntg = np.bincount(batch, minlength=G).astype(np.float32)
    cntinv_g = (1.0 / np.maximum(cntg, 1.0)).astype(np.float32)

    g0s = [int(batch[c * SH]) for c in range(NC)]
    for c in range(NC):
        assert int(batch[(c + 1) * SH - 1]) - g0s[c] < 128, "graph window > 128"

    def idx_pack(lin):
        a = lin.reshape(-1, 16).T
        return np.ascontiguousarray(np.tile(a, (8, 1)))

    per_core = []
    for c in range(NC):
        idx = np.zeros(CH * 128, np.int16)          # pair indices
        dstl = np.full(CH * 128, -1.0, np.float32)  # dst within tile, -1 = pad
        normv = np.zeros(CH * 128, np.float32)
        xg = np.zeros((CH * 128, 8), np.float32)
        for t in range(TPC):
            for p in range(2):
                bkey = (c * TPC + t) * 2 + p
                b0, b1 = block_start[bkey], block_start[bkey + 1]
                n = b1 - b0
                s0 = int(blk_base[t, p]) * 128
                idx[s0:s0 + n] = (src_s[b0:b1] // 2).astype(np.int16)
                dstl[s0:s0 + n] = (dst_s[b0:b1] - (c * SH + t * 128)).astype(
                    np.float32)
                normv[s0:s0 + n] = norm_s[b0:b1]
                xg[s0:s0 + n] = x[src_s[b0:b1]]

        # one-hot scatter matrices [128, CH*128] fp8: oh[p, blk*128 + d] =
        # norm of message (blk, p) if its dst-local == d
        nq = normv.astype(f8).astype(np.float32)
        ohm = np.zeros((CH * 128, 128), np.float32)
        valid = dstl >= 0
        ohm[np.arange(CH * 128)[valid], dstl[valid].astype(np.int64)] = nq[valid]
        ohm = ohm.reshape(CH, 128, 128).transpose(1, 0, 2).reshape(128, CH * 128)

        ns_pad = np.zeros(TPC * 128, np.float32)
        ns_pad[:SH] = norm_self[c * SH:(c + 1) * SH]
        selfoh = np.zeros((128, TPC, 128), np.float32)
        pr = np.arange(128)
        for t in range(TPC):
            selfoh[pr, t, pr] = ns_pad[t * 128:(t + 1) * 128]

        xsl = np.zeros((TPC * 128, 8), np.float32)
        xsl[:SH] = x[c * SH:(c + 1) * SH]

        bl = np.full((TPC * 128,), -1, np.int64)
        bl[:SH] = batch[c * SH:(c + 1) * SH] - g0s[c]
        ohb = np.zeros((TPC * 128, 128), np.float32)
        vb = bl >= 0
        ohb[np.arange(TPC * 128)[vb], bl[vb]] = 1.0
        ohb = ohb.reshape(TPC, 128, 128).transpose(1, 0, 2)
        # (loaded as fp8 below; exact for 0/1 values)

        ig = g0s[c] + np.arange(128)
        cinv = np.where(ig < G, cntinv_g[np.minimum(ig, G - 1)], 0.0)

        per_core.append(dict(
            idx=idx_pack(idx),
            oh=np.ascontiguousarray(ohm).astype(f8),
            selfoh=np.ascontiguousarray(selfoh.reshape(128, TPC * 128)).astype(f8),
            xg=np.ascontiguousarray(
                xg.reshape(CH, 128, 8).transpose(1, 0, 2)).astype(f8),
            xsl=np.ascontiguousarray(
                xsl.reshape(TPC, 128, 8).transpose(1, 0, 2)).astype(f8),
            ohb=np.ascontiguousarray(ohb.reshape(128, TPC * 128)).astype(f8),
            cntinv=cinv.astype(np.float32).reshape(128, 1),
        ))

    struct = dict(CH=CH,
                  ch_uni=[[int(ch_uni[t, p]) for p in range(2)]
                          for t in range(TPC)],
                  blk_base=[[int(blk_base[t, p]) for p in range(2)]
                            for t in range(TPC)],
                  g0s=g0s)
    return per_core, struct


def _build(struct):
    import concourse.bacc as bacc
    import concourse.mybir as mybir
    import concourse.tile as tile
    from concourse.masks import make_identity

    dt = mybir.dt
    AF = mybir.ActivationFunctionType
    OP = mybir.AluOpType

    CH = struct["CH"]
    ch_uni = struct["ch_uni"]
    blk_base = struct["blk_base"]
    g0s = struct["g0s"]
    SEG = (CH + NSEG - 1) // NSEG  # chunks per gather segment

    nc = bacc.Bacc("TRN2", target_bir_lowering=False, debug=False,
                   num_devices=NC, num_swdge_queues=4)

    w1_in = nc.dram_tensor("w1", [8, H1], dt.bfloat16, kind="ExternalInput")
    w2_in = nc.dram_tensor("w2", [H1, H2], dt.bfloat16, kind="ExternalInput")
    w3_in = nc.dram_tensor("w3", [128, 2, H3], dt.bfloat16, kind="ExternalInput")
    wfc_in = nc.dram_tensor("wfc", [128, 4], dt.float32, kind="ExternalInput")
    b1_in = nc.dram_tensor("b1", [1, H1], dt.bfloat16, kind="ExternalInput")
    b2_in = nc.dram_tensor("b2", [1, H2], dt.bfloat16, kind="ExternalInput")
    b3_in = nc.dram_tensor("b3", [1, H3], dt.bfloat16, kind="ExternalInput")
    bfc_in = nc.dram_tensor("bfc", [1, 1], dt.float32, kind="ExternalInput")

    idx_in = nc.dram_tensor("idx", [128, CH * 8], dt.int16, kind="ExternalInput")
    oh_in = nc.dram_tensor("oh", [128, CH * 128], dt.float8e4,
                           kind="ExternalInput")
    selfoh_in = nc.dram_tensor("selfoh", [128, TPC * 128], dt.float8e4,
                               kind="ExternalInput")
    xg_in = nc.dram_tensor("xg", [128, CH * 8], dt.float8e4,
                           kind="ExternalInput")
    xsl_in = nc.dram_tensor("xsl", [128, TPC * 8], dt.float8e4,
                            kind="ExternalInput")
    ohb_in = nc.dram_tensor("ohb", [128, TPC * 128], dt.float8e4,
                            kind="ExternalInput")
    cinv_in = nc.dram_tensor("cntinv", [128, 1], dt.float32, kind="ExternalInput")
    out_ext = nc.dram_tensor("out", [G, 1], dt.float32, kind="ExternalOutput")

    with tile.TileContext(nc) as tc:
        with tc.tile_pool(name="const", bufs=1) as cp, \
             tc.tile_pool(name="meta", bufs=1) as mp, \
             tc.tile_pool(name="gseg", bufs=NSEG) as gp, \
             tc.tile_pool(name="work", bufs=3) as wp, \
             tc.tile_pool(name="slab", bufs=1) as slp, \
             tc.tile_pool(name="pps", bufs=1, space="PSUM") as pps, \
             tc.tile_pool(name="dram", bufs=1, space="DRAM") as dram:

            def load(pool, t_in, shape, dtype, tag):
                t = pool.tile(shape, dtype, tag=tag)
                nc.sync.dma_start(t[:], t_in[:])
                return t

            w1_sb = load(cp, w1_in, [8, H1], dt.bfloat16, "w1")
            w2_sb = load(cp, w2_in, [H1, H2], dt.bfloat16, "w2")
            w3_sb = load(cp, w3_in, [128, 2, H3], dt.bfloat16, "w3")
            wfc_sb = load(cp, wfc_in, [128, 4], dt.float32, "wfc")
            b1_sb = load(cp, b1_in, [1, H1], dt.bfloat16, "b1")
            b2_sb = load(cp, b2_in, [1, H2], dt.bfloat16, "b2")
            b3_sb = load(cp, b3_in, [1, H3], dt.bfloat16, "b3")
            bfc_sb = load(cp, bfc_in, [1, 1], dt.float32, "bfc")
            idx_sb = load(mp, idx_in, [128, CH * 8], dt.int16, "idx")
            # piecewise loads: tile 0's matmuls gate only on the first slice
            oh_sb = mp.tile([128, CH * 128], dt.float8e4, tag="oh")
            ohq = (CH * 128) // 4
            for _q in range(4):
                _e = CH * 128 if _q == 3 else (_q + 1) * ohq
                nc.sync.dma_start(oh_sb[:, _q * ohq:_e],
                                  oh_in[:, _q * ohq:_e])
            selfoh_sb = mp.tile([128, TPC, 128], dt.float8e4, tag="selfoh")
            soq = TPC // 4
            for _q in range(4):
                _e = TPC if _q == 3 else (_q + 1) * soq
                nc.sync.dma_start(
                    selfoh_sb[:, _q * soq:_e, :],
                    selfoh_in[:].rearrange("p (t f) -> p t f", f=128)
                    [:, _q * soq:_e, :])
            xg_sb = load(mp, xg_in, [128, CH, 8], dt.float8e4, "xg")
            xsl_sb = load(mp, xsl_in, [128, TPC, 8], dt.float8e4, "xsl")
            ohb_sb = load(mp, ohb_in, [128, TPC, 128], dt.float8e4, "ohb")
            cinv_sb = load(mp, cinv_in, [128, 1], dt.float32, "cinv")

            ones_bf = cp.tile([1, 128], dt.bfloat16, tag="ones_bf")
            nc.gpsimd.memset(ones_bf[:], 1.0)
            ones_f32 = cp.tile([1, 128], dt.float32, tag="ones_f32")
            nc.gpsimd.memset(ones_f32[:], 1.0)
            ident = cp.tile([128, 128], dt.float32, tag="ident")
            make_identity(nc, ident[:])
            ident_bf = cp.tile([128, 128], dt.bfloat16, tag="identbf")
            nc.vector.tensor_copy(ident_bf[:], ident[:])

            h1sl = slp.tile([128, TPC, H1], dt.float8e4, tag="h1sl")
            h2sl = slp.tile([128, TPC, H2], dt.float8e4, tag="h2sl")

            h1_own = dram.tile([SH, H1], dt.float8e4, tag="h1own")
            h1_all = dram.tile([N, H1], dt.float8e4, tag="h1all",
                               addr_space="Shared")
            h2_own = dram.tile([SH, H2], dt.float8e4, tag="h2own")
            h2_all = dram.tile([N, H2], dt.float8e4, tag="h2all",
                               addr_space="Shared")
            pool_own = dram.tile([128, H3], dt.bfloat16, tag="plown")
            pool_all = dram.tile([NC * 128, H3], dt.bfloat16, tag="plall",
                                 addr_space="Shared")

            def tile_blocks(t):
                """[(blk, par), ...] for tile t in stream order."""
                out = []
                for p in range(2):
                    for k in range(ch_uni[t][p]):
                        out.append((blk_base[t][p] + k, p))
                return out

            def ag(in_ap, out_t):
                nc.gpsimd.collective_compute(
                    "AllGather", mybir.AluOpType.bypass,
                    replica_groups=[list(range(NC))],
                    ins=[in_ap.opt() if hasattr(in_ap, 'opt') else in_ap],
                    outs=[out_t.opt()])

            def ag_chunked(own_t, all_t, sh, f):
                """AllGather own [sh, f] -> all [NC*sh, f] in AGCHUNK pieces
                so early pieces overlap the producing layer's tail."""
                if AGCHUNK <= 1:
                    ag(own_t[:, :], all_t)
                    return
                view = all_t[:].rearrange("(c r) f -> c r f", c=NC)
                bnd = [0]
                step = ((sh // AGCHUNK) // 128) * 128
                for k in range(1, AGCHUNK):
                    bnd.append(step * k)
                bnd.append(sh)
                for k in range(AGCHUNK):
                    r0, r1 = bnd[k], bnd[k + 1]
                    nc.gpsimd.collective_compute(
                        "AllGather", mybir.AluOpType.bypass,
                        replica_groups=[list(range(NC))],
                        ins=[own_t[r0:r1, :]],
                        outs=[view[:, r0:r1, :]])

            # ---------------- Layer 1 (host-gathered stream) ----------------
            with tc.tile_pool(name="ps1", bufs=1, space="PSUM") as ps1:
                for t in range(TPC):
                    rows = min(128, SH - t * 128)
                    blocks = tile_blocks(t)
                    aggx_ps = ps1.tile([8, 128], dt.float32, tag="aggx", bufs=2)
                    nc.tensor.matmul(aggx_ps[:], lhsT=xsl_sb[:, t, :],
                                     rhs=selfoh_sb[:, t, :],
                                     start=True, stop=False)
                    for i, (blk, p) in enumerate(blocks):
                        nc.tensor.matmul(aggx_ps[:], lhsT=xg_sb[:, blk, :],
                                         rhs=oh_sb[:, blk * 128:(blk + 1) * 128],
                                         start=False, stop=(i == len(blocks) - 1))
                    aggx_sb = wp.tile([8, 128], dt.bfloat16, tag="aggxsb",
                                      bufs=2)
                    nc.scalar.activation(aggx_sb[:], aggx_ps[:], AF.Copy)
                    h1_ps = ps1.tile([128, H1], dt.float32, tag="h1ps", bufs=2)
                    nc.tensor.matmul(h1_ps[:], lhsT=ones_bf[:], rhs=b1_sb[:],
                                     start=True, stop=False)
                    nc.tensor.matmul(h1_ps[:], lhsT=aggx_sb[:], rhs=w1_sb[:],
                                     start=False, stop=True)
                    # relu -> fp8 slab; DMA the table row straight from it
                    nc.scalar.activation(h1sl[:, t, :], h1_ps[:], AF.Relu)
                    nc.sync.dma_start(h1_own[t * 128:t * 128 + rows, :],
                                      h1sl[:rows, t, :])

            ag_chunked(h1_own, h1_all, SH, H1)

            # ---------------- Layers 2/3 (gather + scatter matmuls) --------
            def do_layer(tab_pairs, elem2, fcn, w_rhs, b_sb, slab_in, slab_out,
                         own_out, pool_ps, sem_name):
                # A stalled semaphore wait serializes the gpsimd dispatch
                # pipeline, so absorb the collective-done wait with a tiny
                # dummy read; the real gathers then dispatch back-to-back
                # and desc-gen runs 4-way parallel across SWDGE queues.
                scrap = wp.tile([128, elem2], dt.float8e4, tag="scrap")
                nc.gpsimd.dma_start(scrap[:1, :], tab_pairs[0:1, :])
                seg_tiles = []
                for s in range(NSEG):
                    # one shared slot pool (sized for L3's 512B rows); L2's
                    # 256B rows use the contiguous first part of each slot
                    nch = min(SEG, CH - s * SEG)
                    slot = gp.tile([128, SEG * 2 * H2], dt.float8e4,
                                   tag="gseg")
                    t_ = slot[:, :SEG * elem2].rearrange(
                        "p (c e) -> p c e", e=elem2)
                    nc.gpsimd.dma_gather(
                        t_[:, :nch, :], tab_pairs,
                        idx_sb[:, s * SEG * 8:(s * SEG + nch) * 8],
                        nch * 128, nch * 128, elem2, queue_num=(s % 4),
                        single_packet=False)
                    seg_tiles.append(t_)

                def seg(s):
                    return seg_tiles[s]

                fout = H2 if fcn == 1 else H3
                with tc.tile_pool(name=f"psl{fcn}", bufs=1, space="PSUM") as psl:
                    for t in range(TPC):
                        rows = min(128, SH - t * 128)
                        blocks = tile_blocks(t)
                        aggs = [psl.tile([128, 128], dt.float32, tag=f"agg{fc}",
                                         name=f"agg{fc}", bufs=2)
                                for fc in range(fcn)]
                        for fc in range(fcn):
                            nc.tensor.matmul(
                                aggs[fc][:],
                                lhsT=slab_in[:, t, fc * 128:(fc + 1) * 128],
                                rhs=selfoh_sb[:, t, :], start=True, stop=False)
                        for i, (blk, p) in enumerate(blocks):
                            sgt = seg(blk // SEG)
                            col = blk % SEG
                            F = elem2 // 2
                            for fc in range(fcn):
                                nc.tensor.matmul(
                                    aggs[fc][:],
                                    lhsT=sgt[:, col,
                                             p * F + fc * 128:
                                             p * F + (fc + 1) * 128],
                                    rhs=oh_sb[:, blk * 128:(blk + 1) * 128],
                                    start=False, stop=(i == len(blocks) - 1))
                        agg_sbs = []
                        for fc in range(fcn):
                            a = wp.tile([128, 128], dt.bfloat16, tag=f"asb{fc}",
                                        name=f"asb{fc}")
                            if fc % 2 == 0:
                                nc.scalar.activation(a[:], aggs[fc][:], AF.Copy)
                            else:
                                nc.vector.tensor_copy(a[:], aggs[fc][:])
                            agg_sbs.append(a)
                        h_ps = psl.tile([128, fout], dt.float32, tag="hps",
                                        bufs=2)
                        nc.tensor.matmul(h_ps[:], lhsT=ones_bf[:], rhs=b_sb[:],
                                         start=True, stop=False)
                        for fc in range(fcn):
                            nc.tensor.matmul(h_ps[:], lhsT=agg_sbs[fc][:],
                                             rhs=w_rhs(fc), start=False,
                                             stop=(fc == fcn - 1))
                        if slab_out is not None:
                            nc.scalar.activation(slab_out[:, t, :], h_ps[:],
                                                 AF.Relu)
                            nc.sync.dma_start(own_out[t * 128:t * 128 + rows, :],
                                              slab_out[:rows, t, :])
                        else:
                            h3_sb = wp.tile([128, fout], dt.float8e4,
                                            tag="h3sb")
                            nc.scalar.activation(h3_sb[:], h_ps[:], AF.Relu)
                            nc.tensor.matmul(pool_ps[:], lhsT=ohb_sb[:, t, :],
                                             rhs=h3_sb[:], start=(t == 0),
                                             stop=(t == TPC - 1))

            do_layer(h1_all[:, :].rearrange("(n two) f -> n (two f)", two=2),
                     2 * H1, 1, lambda fc: w2_sb[:], b2_sb, h1sl, h2sl,
                     h2_own, None, "gsem2")
            ag_chunked(h2_own, h2_all, SH, H2)

            pool_ps = pps.tile([128, H3], dt.float32)
            do_layer(h2_all[:, :].rearrange("(n two) f -> n (two f)", two=2),
                     2 * H2, 2, lambda fc: w3_sb[:, fc, :], b3_sb, h2sl, None,
                     None, pool_ps, "gsem3")

            pool_sb = wp.tile([128, H3], dt.float32, tag="poolsb")
            nc.vector.tensor_scalar(pool_sb[:], pool_ps[:], cinv_sb[:, :1],
                                    None, OP.mult)
            pool_bf = wp.tile([128, H3], dt.bfloat16, tag="poolbf")
            nc.scalar.activation(pool_bf[:], pool_sb[:], AF.Copy)
            nc.sync.dma_start(pool_own[:], pool_bf[:])
            ag(pool_own[:, :], pool_all)

            # ---------------- FC head (replicated) --------------------------
            with tc.tile_pool(name="psf", bufs=2, space="PSUM") as psf:
                poolT = []
                for fc in range(4):
                    pt = cp.tile([128, G], dt.float32, tag=f"poolT{fc}")
                    nc.gpsimd.memset(pt[:], 0.0)
                    poolT.append(pt)
                for c in range(NC):
                    pc_sb = wp.tile([128, H3], dt.bfloat16, tag="pc", bufs=4)
                    nc.sync.dma_start(pc_sb[:], pool_all[c * 128:(c + 1) * 128, :])
                    wcols = min(128, G - g0s[c])
                    for fc in range(4):
                        tp_ps = psf.tile([128, 128], dt.bfloat16, tag="tp",
                                         bufs=4)
                        nc.tensor.transpose(tp_ps[:],
                                            pc_sb[:, fc * 128:(fc + 1) * 128],
                                            ident_bf[:])
                        sl = poolT[fc][:, g0s[c]:g0s[c] + wcols]
                        nc.vector.tensor_tensor(sl, sl, tp_ps[:, :wcols], OP.add)
                for gh in range(2):
                    fc_ps = psf.tile([128, 1], dt.float32, tag="fcps")
                    nc.tensor.matmul(fc_ps[:], lhsT=ones_f32[:], rhs=bfc_sb[:],
                                     start=True, stop=False)
                    for fc in range(4):
                        nc.tensor.matmul(fc_ps[:],
                                         lhsT=poolT[fc][:, gh * 128:(gh + 1) * 128],
                                         rhs=wfc_sb[:, fc:fc + 1],
                                         start=False, stop=(fc == 3))
                    o_sb = wp.tile([128, 1], dt.float32, tag="osb")
                    nc.scalar.activation(o_sb[:], fc_ps[:], AF.Sigmoid)
                    nc.sync.dma_start(out_ext[gh * 128:(gh + 1) * 128, :], o_sb[:])

    nc.compile()
    return nc


def _install_profile_hook():
    import importlib
    try:
        importlib.import_module("antenv.axon_hooks")
        return
    except ImportError:
        pass
    import types
    import ctypes
    import contextlib
    so_path = "/opt/axon/libaxon_pjrt.so"
    mod = types.ModuleType("antenv.axon_hooks")
    _state = {"hook": None}

    def set_axon_ntff_profile_hook(h):
        _state["hook"] = h

    def get_axon_ntff_profile_hook():
        if _state["hook"] is None and os.path.exists(so_path):
            lib = ctypes.CDLL(so_path)
            if hasattr(lib, "axon_start_nrt_profile"):
                lib.axon_start_nrt_profile.argtypes = [
                    ctypes.POINTER(ctypes.c_int64), ctypes.c_size_t]
                lib.axon_start_nrt_profile.restype = ctypes.c_int64
                lib.axon_stop_nrt_profile.argtypes = [ctypes.c_char_p]
                lib.axon_stop_nrt_profile.restype = ctypes.c_int64

                @contextlib.contextmanager
                def _hook(output_dir, device_ids):
                    import jax
                    jax.devices()
                    if device_ids:
                        ids = (ctypes.c_int64 * len(device_ids))(*device_ids)
                        rc = lib.axon_start_nrt_profile(ids, len(device_ids))
                    else:
                        rc = lib.axon_start_nrt_profile(None, 0)
                    if rc != 0:
                        raise RuntimeError(f"axon_start_nrt_profile rc={rc}")
                    try:
                        yield
                    finally:
                        n = lib.axon_stop_nrt_profile(str(output_dir).encode())
                        print(f"profile: {n} file(s) written to {output_dir}")

                _state["hook"] = _hook
        return _state["hook"]

    mod.set_axon_ntff_profile_hook = set_axon_ntff_profile_hook
    mod.get_axon_ntff_profile_hook = get_axon_ntff_profile_hook
    sys.modules["antenv.axon_hooks"] = mod


def kernel(**inputs):
    global LAST_EXEC_NS
    from concourse.bass_utils import run_bass_kernel_spmd

    per_core, struct = _prep(inputs["x"], inputs["edge_index"],
                             inputs["edge_weight"], inputs["batch"])

    key = (struct["CH"], tuple(map(tuple, struct["ch_uni"])),
           tuple(struct["g0s"]))
    if key not in _CACHE:
        _CACHE[key] = _build(struct)
    nc = _CACHE[key]

    W1 = np.asarray(inputs["W1"], np.float32)
    W2 = np.asarray(inputs["W2"], np.float32)
    W3 = np.asarray(inputs["W3"], np.float32)
    Wfc = np.asarray(inputs["Wfc"], np.float32)
    shared = dict(
        w1=W1.astype(bf16),
        w2=W2.astype(bf16),
        w3=np.ascontiguousarray(
            W3.reshape(2, 128, H3).transpose(1, 0, 2)).astype(bf16),
        wfc=np.ascontiguousarray(Wfc.reshape(4, 128).T).astype(np.float32),
        b1=np.asarray(inputs["b1"], np.float32).reshape(1, H1).astype(bf16),
        b2=np.asarray(inputs["b2"], np.float32).reshape(1, H2).astype(bf16),
        b3=np.asarray(inputs["b3"], np.float32).reshape(1, H3).astype(bf16),
        bfc=np.asarray(inputs["bfc"], np.float32).reshape(1, 1),
    )
    in_maps = [{**shared, **pc} for pc in per_core]

    if TRACE:
        _install_profile_hook()
    res = run_bass_kernel_spmd(nc, in_maps, list(range(NC)), trace=TRACE)
    LAST_EXEC_NS = res.exec_time_ns
    return res.results[0]["out"]



# revision 40
# speedup vs baseline: 1.1082x; 1.1082x over previous
"""GCN (3-layer + global mean pool + FC/sigmoid) on 8 Trainium2 NeuronCores, v2.

Aggregate-first node-sharded design. One canonical per-core edge order —
sorted by (dst tile, src parity, src) — is shared by all three layers: the
same fp8 one-hot scatter matrices drive layer 1/2/3 chunk matmuls, and
self-loops use per-tile bf16 diagonal matmuls from SBUF slabs.

Layer 1 needs no device gather at all: the host pre-gathers x[src] into the
canonical stream (xg, fp8). Layers 2/3 gather h1/h2 rows from fp8
"pair" tables ([N/2, 2F], one pair per 256B+ row, int16 pair indices) that
are exchanged with a single AllGather each. Tables, one-hot norms, and the
gathered message blocks are all fp8 (validated ~1e-3 max-rel); slabs,
weights, and the pooling path stay bf16/f32.
"""
import sys
import os

for _p in ("/opt/trn_rl_repo", "/root/.axon_site/_ro/trn_rl_repo"):
    if os.path.isdir(_p) and _p not in sys.path:
        sys.path.append(_p)

import numpy as np
import ml_dtypes

bf16 = ml_dtypes.bfloat16
f8 = ml_dtypes.float8_e4m3

N = 50000
E = 150000
G = 256
NC = 8
SH = N // NC             # 6250 nodes per core
TPC = (SH + 127) // 128  # 49 tiles per core
H1, H2, H3 = 128, 256, 512
SEG = 8                   # chunks per gather (1024 descs = single-packet cap)
GBUFS = int(os.environ.get('KGBUFS', '20'))

TRACE = False
LAST_EXEC_NS = None
_CACHE = {}
AGCHUNK = int(os.environ.get('KAGCHUNK', '1'))


def _prep(x, edge_index, edge_weight, batch):
    """Host-side graph preprocessing -> per-core input arrays + structure."""
    x = np.asarray(x, np.float32)
    ei = np.asarray(edge_index)
    ew = np.asarray(edge_weight, np.float32)
    batch = np.asarray(batch).astype(np.int64)

    src = ei[0].astype(np.int64)
    dst = ei[1].astype(np.int64)
    deg = np.bincount(dst, weights=ew, minlength=N).astype(np.float32) + 1.0
    dinv = (1.0 / np.sqrt(deg)).astype(np.float32)
    norm = (dinv[src] * ew * dinv[dst]).astype(np.float32)
    norm_self = (dinv * dinv).astype(np.float32)

    core = dst // SH
    tile_l = (dst % SH) // 128
    par = src % 2
    key = (core * TPC + tile_l) * 2 + par
    order = np.lexsort((src, key))
    src_s, dst_s, norm_s, key_s = src[order], dst[order], norm[order], key[order]

    cnt = np.bincount(key, minlength=NC * TPC * 2).reshape(NC, TPC, 2)
    ch_uni = np.ceil(cnt / 128).astype(np.int64).max(axis=0)   # [TPC, 2]
    CH = int(ch_uni.sum())
    # global block index of (tile, parity, k): blocks laid out tile-major
    blk_base = np.zeros((TPC, 2), np.int64)
    running = 0
    for t in range(TPC):
        for p in range(2):
            blk_base[t, p] = running
            running += ch_uni[t, p]

    block_start = np.zeros(NC * TPC * 2 + 1, np.int64)
    block_start[1:] = np.cumsum(cnt.reshape(-1))

    cntg = np.bincount(batch, minlength=G).astype(np.float32)
    cntinv_g = (1.0 / np.maximum(cntg, 1.0)).astype(np.float32)

    g0s = [int(batch[c * SH]) for c in range(NC)]
    for c in range(NC):
        assert int(batch[(c + 1) * SH - 1]) - g0s[c] < 128, "graph window > 128"

    def idx_pack(lin):
        a = lin.reshape(-1, 16).T
        return np.ascontiguousarray(np.tile(a, (8, 1)))

    per_core = []
    for c in range(NC):
        idx = np.zeros(CH * 128, np.int16)          # pair indices
        dstl = np.full(CH * 128, -1.0, np.float32)  # dst within tile, -1 = pad
        normv = np.zeros(CH * 128, np.float32)
        xg = np.zeros((CH * 128, 8), np.float32)
        for t in range(TPC):
            for p in range(2):
                bkey = (c * TPC + t) * 2 + p
                b0, b1 = block_start[bkey], block_start[bkey + 1]
                n = b1 - b0
                s0 = int(blk_base[t, p]) * 128
                idx[s0:s0 + n] = (src_s[b0:b1] // 2).astype(np.int16)
                dstl[s0:s0 + n] = (dst_s[b0:b1] - (c * SH + t * 128)).astype(
                    np.float32)
                normv[s0:s0 + n] = norm_s[b0:b1]
                xg[s0:s0 + n] = x[src_s[b0:b1]]

        # one-hot scatter matrices [128, CH*128] fp8: oh[p, blk*128 + d] =
        # norm of message (blk, p) if its dst-local == d
        nq = normv.astype(f8).astype(np.float32)
        ohm = np.zeros((CH * 128, 128), np.float32)
        valid = dstl >= 0
        ohm[np.arange(CH * 128)[valid], dstl[valid].astype(np.int64)] = nq[valid]
        ohm = ohm.reshape(CH, 128, 128).transpose(1, 0, 2).reshape(128, CH * 128)

        ns_pad = np.zeros(TPC * 128, np.float32)
        ns_pad[:SH] = norm_self[c * SH:(c + 1) * SH]
        selfoh = np.zeros((128, TPC, 128), np.float32)
        pr = np.arange(128)
        for t in range(TPC):
            selfoh[pr, t, pr] = ns_pad[t * 128:(t + 1) * 128]

        xsl = np.zeros((TPC * 128, 8), np.float32)
        xsl[:SH] = x[c * SH:(c + 1) * SH]

        bl = np.full((TPC * 128,), -1, np.int64)
        bl[:SH] = batch[c * SH:(c + 1) * SH] - g0s[c]
        ohb = np.zeros((TPC * 128, 128), np.float32)
        vb = bl >= 0
        ohb[np.arange(TPC * 128)[vb], bl[vb]] = 1.0
        ohb = ohb.reshape(TPC, 128, 128).transpose(1, 0, 2)
        # (loaded as fp8 below; exact for 0/1 values)

        ig = g0s[c] + np.arange(128)
        cinv = np.where(ig < G, cntinv_g[np.minimum(ig, G - 1)], 0.0)

        per_core.append(dict(
            idx=idx_pack(idx),
            oh=np.ascontiguousarray(ohm).astype(f8),
            selfoh=np.ascontiguousarray(selfoh.reshape(128, TPC * 128)).astype(f8),
            xg=np.ascontiguousarray(
                xg.reshape(CH, 128, 8).transpose(1, 0, 2)).astype(f8),
            xsl=np.ascontiguousarray(
                xsl.reshape(TPC, 128, 8).transpose(1, 0, 2)).astype(f8),
            ohb=np.ascontiguousarray(ohb.reshape(128, TPC * 128)).astype(f8),
            cntinv=cinv.astype(np.float32).reshape(128, 1),
        ))

    struct = dict(CH=CH,
                  ch_uni=[[int(ch_uni[t, p]) for p in range(2)]
                          for t in range(TPC)],
                  blk_base=[[int(blk_base[t, p]) for p in range(2)]
                            for t in range(TPC)],
                  g0s=g0s)
    return per_core, struct


def _build(struct):
    import concourse.bacc as bacc
    import concourse.mybir as mybir
    import concourse.tile as tile
    from concourse.masks import make_identity

    dt = mybir.dt
    AF = mybir.ActivationFunctionType
    OP = mybir.AluOpType

    CH = struct["CH"]
    ch_uni = struct["ch_uni"]
    blk_base = struct["blk_base"]
    g0s = struct["g0s"]
    NSEG = (CH + SEG - 1) // SEG

    nc = bacc.Bacc("TRN2", target_bir_lowering=False, debug=False,
                   num_devices=NC, num_swdge_queues=4)

    w1_in = nc.dram_tensor("w1", [8, H1], dt.bfloat16, kind="ExternalInput")
    w2_in = nc.dram_tensor("w2", [H1, H2], dt.bfloat16, kind="ExternalInput")
    w3_in = nc.dram_tensor("w3", [128, 2, H3], dt.bfloat16, kind="ExternalInput")
    wfc_in = nc.dram_tensor("wfc", [128, 4], dt.float32, kind="ExternalInput")
    b1_in = nc.dram_tensor("b1", [1, H1], dt.bfloat16, kind="ExternalInput")
    b2_in = nc.dram_tensor("b2", [1, H2], dt.bfloat16, kind="ExternalInput")
    b3_in = nc.dram_tensor("b3", [1, H3], dt.bfloat16, kind="ExternalInput")
    bfc_in = nc.dram_tensor("bfc", [1, 1], dt.float32, kind="ExternalInput")

    idx_in = nc.dram_tensor("idx", [128, CH * 8], dt.int16, kind="ExternalInput")
    oh_in = nc.dram_tensor("oh", [128, CH * 128], dt.float8e4,
                           kind="ExternalInput")
    selfoh_in = nc.dram_tensor("selfoh", [128, TPC * 128], dt.float8e4,
                               kind="ExternalInput")
    xg_in = nc.dram_tensor("xg", [128, CH * 8], dt.float8e4,
                           kind="ExternalInput")
    xsl_in = nc.dram_tensor("xsl", [128, TPC * 8], dt.float8e4,
                            kind="ExternalInput")
    ohb_in = nc.dram_tensor("ohb", [128, TPC * 128], dt.float8e4,
                            kind="ExternalInput")
    cinv_in = nc.dram_tensor("cntinv", [128, 1], dt.float32, kind="ExternalInput")
    out_ext = nc.dram_tensor("out", [G, 1], dt.float32, kind="ExternalOutput")

    with tile.TileContext(nc) as tc:
        with tc.tile_pool(name="const", bufs=1) as cp, \
             tc.tile_pool(name="meta", bufs=1) as mp, \
             tc.tile_pool(name="gseg", bufs=GBUFS) as gp, \
             tc.tile_pool(name="work", bufs=3) as wp, \
             tc.tile_pool(name="slab", bufs=1) as slp, \
             tc.tile_pool(name="pps", bufs=1, space="PSUM") as pps, \
             tc.tile_pool(name="dram", bufs=1, space="DRAM") as dram:

            def load(pool, t_in, shape, dtype, tag):
                t = pool.tile(shape, dtype, tag=tag)
                nc.sync.dma_start(t[:], t_in[:])
                return t

            w1_sb = load(cp, w1_in, [8, H1], dt.bfloat16, "w1")
            w2_sb = load(cp, w2_in, [H1, H2], dt.bfloat16, "w2")
            w3_sb = load(cp, w3_in, [128, 2, H3], dt.bfloat16, "w3")
            wfc_sb = load(cp, wfc_in, [128, 4], dt.float32, "wfc")
            b1_sb = load(cp, b1_in, [1, H1], dt.bfloat16, "b1")
            b2_sb = load(cp, b2_in, [1, H2], dt.bfloat16, "b2")
            b3_sb = load(cp, b3_in, [1, H3], dt.bfloat16, "b3")
            bfc_sb = load(cp, bfc_in, [1, 1], dt.float32, "bfc")
            idx_sb = load(mp, idx_in, [128, CH * 8], dt.int16, "idx")
            # piecewise loads: tile 0's matmuls gate only on the first slice
            oh_sb = mp.tile([128, CH * 128], dt.float8e4, tag="oh")
            ohq = (CH * 128) // 4
            for _q in range(4):
                _e = CH * 128 if _q == 3 else (_q + 1) * ohq
                nc.sync.dma_start(oh_sb[:, _q * ohq:_e],
                                  oh_in[:, _q * ohq:_e])
            selfoh_sb = mp.tile([128, TPC, 128], dt.float8e4, tag="selfoh")
            soq = TPC // 4
            for _q in range(4):
                _e = TPC if _q == 3 else (_q + 1) * soq
                nc.sync.dma_start(
                    selfoh_sb[:, _q * soq:_e, :],
                    selfoh_in[:].rearrange("p (t f) -> p t f", f=128)
                    [:, _q * soq:_e, :])
            xg_sb = load(mp, xg_in, [128, CH, 8], dt.float8e4, "xg")
            xsl_sb = load(mp, xsl_in, [128, TPC, 8], dt.float8e4, "xsl")
            ohb_sb = load(mp, ohb_in, [128, TPC, 128], dt.float8e4, "ohb")
            cinv_sb = load(mp, cinv_in, [128, 1], dt.float32, "cinv")

            ones_bf = cp.tile([1, 128], dt.bfloat16, tag="ones_bf")
            nc.gpsimd.memset(ones_bf[:], 1.0)
            ones_f32 = cp.tile([1, 128], dt.float32, tag="ones_f32")
            nc.gpsimd.memset(ones_f32[:], 1.0)
            ident = cp.tile([128, 128], dt.float32, tag="ident")
            make_identity(nc, ident[:])
            ident_bf = cp.tile([128, 128], dt.bfloat16, tag="identbf")
            nc.vector.tensor_copy(ident_bf[:], ident[:])

            h1sl = slp.tile([128, TPC, H1], dt.float8e4, tag="h1sl")
            h2sl = slp.tile([128, TPC, H2], dt.float8e4, tag="h2sl")

            pool_dmas = []  # 1-elem views of each pool-DMA output, lane order
            sink_sb = cp.tile([1, 4], dt.float8e4, tag="sink")

            h1_own = dram.tile([SH, H1], dt.float8e4, tag="h1own")
            h1_all = dram.tile([N, H1], dt.float8e4, tag="h1all",
                               addr_space="Shared")
            h2_own = dram.tile([SH, H2], dt.float8e4, tag="h2own")
            h2_all = dram.tile([N, H2], dt.float8e4, tag="h2all",
                               addr_space="Shared")
            pool_own = dram.tile([128, H3], dt.bfloat16, tag="plown")
            pool_all = dram.tile([NC * 128, H3], dt.bfloat16, tag="plall",
                                 addr_space="Shared")

            def tile_blocks(t):
                """[(blk, par), ...] for tile t in stream order."""
                out = []
                for p in range(2):
                    for k in range(ch_uni[t][p]):
                        out.append((blk_base[t][p] + k, p))
                return out

            def ag(in_ap, out_t):
                nc.gpsimd.collective_compute(
                    "AllGather", mybir.AluOpType.bypass,
                    replica_groups=[list(range(NC))],
                    ins=[in_ap.opt() if hasattr(in_ap, 'opt') else in_ap],
                    outs=[out_t.opt()])

            def ag_chunked(own_t, all_t, sh, f):
                """AllGather own [sh, f] -> all [NC*sh, f] in AGCHUNK pieces
                so early pieces overlap the producing layer's tail."""
                if AGCHUNK <= 1:
                    ag(own_t[:, :], all_t)
                    return
                view = all_t[:].rearrange("(c r) f -> c r f", c=NC)
                bnd = [0]
                step = ((sh // AGCHUNK) // 128) * 128
                for k in range(1, AGCHUNK):
                    bnd.append(step * k)
                bnd.append(sh)
                for k in range(AGCHUNK):
                    r0, r1 = bnd[k], bnd[k + 1]
                    nc.gpsimd.collective_compute(
                        "AllGather", mybir.AluOpType.bypass,
                        replica_groups=[list(range(NC))],
                        ins=[own_t[r0:r1, :]],
                        outs=[view[:, r0:r1, :]])

            # ---------------- Layer 1 (host-gathered stream) ----------------
            with tc.tile_pool(name="ps1", bufs=1, space="PSUM") as ps1:
                for t in range(TPC):
                    rows = min(128, SH - t * 128)
                    blocks = tile_blocks(t)
                    aggx_ps = ps1.tile([8, 128], dt.float32, tag="aggx", bufs=2)
                    nc.tensor.matmul(aggx_ps[:], lhsT=xsl_sb[:, t, :],
                                     rhs=selfoh_sb[:, t, :],
                                     start=True, stop=False)
                    for i, (blk, p) in enumerate(blocks):
                        nc.tensor.matmul(aggx_ps[:], lhsT=xg_sb[:, blk, :],
                                         rhs=oh_sb[:, blk * 128:(blk + 1) * 128],
                                         start=False, stop=(i == len(blocks) - 1))
                    aggx_sb = wp.tile([8, 128], dt.bfloat16, tag="aggxsb",
                                      bufs=2)
                    nc.scalar.activation(aggx_sb[:], aggx_ps[:], AF.Copy)
                    h1_ps = ps1.tile([128, H1], dt.float32, tag="h1ps", bufs=2)
                    nc.tensor.matmul(h1_ps[:], lhsT=ones_bf[:], rhs=b1_sb[:],
                                     start=True, stop=False)
                    nc.tensor.matmul(h1_ps[:], lhsT=aggx_sb[:], rhs=w1_sb[:],
                                     start=False, stop=True)
                    # relu -> fp8 slab; DMA the table row straight from it
                    nc.scalar.activation(h1sl[:, t, :], h1_ps[:], AF.Relu)
                    nc.sync.dma_start(h1_own[t * 128:t * 128 + rows, :],
                                      h1sl[:rows, t, :])

            ag_chunked(h1_own, h1_all, SH, H1)

            # ---------------- Layers 2/3 (gather + scatter matmuls) --------
            def do_layer(tab_pairs, elem2, fcn, w_rhs, b_sb, slab_in, slab_out,
                         own_out, pool_ps, sem_name):
                # A gpsimd instruction carrying a semaphore wait serializes
                # the Q7 dispatch pipeline until it retires; waitless ones
                # pipeline freely (4-way desc-gen across SWDGE queues). So
                # keep the gathers waitless: a dummy 1-row table read absorbs
                # the collective-done wait, and before each gather whose
                # DMASW lane is being recycled (every 8th pool-DMA) a 1-elem
                # gpsimd copy reads the lane predecessor's output so Tile
                # attaches the recycle wait to the copy and elides it on the
                # gather itself.
                scrap = wp.tile([128, elem2], dt.float8e4, tag="scrap")
                nc.gpsimd.dma_start(scrap[:1, :], tab_pairs[0:1, :])
                pool_dmas.append(scrap[:1, 0:1])
                seg_tiles = []
                for s in range(NSEG):
                    k = len(pool_dmas)
                    if k >= 8:
                        nc.gpsimd.tensor_copy(sink_sb[:1, 0:1],
                                              pool_dmas[k - 8])
                    # one shared slot pool (sized for L3's 512B rows); L2's
                    # 256B rows use the contiguous first part of each slot
                    nch = min(SEG, CH - s * SEG)
                    slot = gp.tile([128, SEG * 2 * H2], dt.float8e4,
                                   tag="gseg")
                    t_ = slot[:, :SEG * elem2].rearrange(
                        "p (c e) -> p c e", e=elem2)
                    nc.gpsimd.dma_gather(
                        t_[:, :nch, :], tab_pairs,
                        idx_sb[:, s * SEG * 8:(s * SEG + nch) * 8],
                        nch * 128, nch * 128, elem2, queue_num=(s % 4))
                    pool_dmas.append(t_[:1, 0, 0:1])
                    seg_tiles.append(t_)

                def seg(s):
                    return seg_tiles[s]

                fout = H2 if fcn == 1 else H3
                with tc.tile_pool(name=f"psl{fcn}", bufs=1, space="PSUM") as psl:
                    for t in range(TPC):
                        rows = min(128, SH - t * 128)
                        blocks = tile_blocks(t)
                        aggs = [psl.tile([128, 128], dt.float32, tag=f"agg{fc}",
                                         name=f"agg{fc}", bufs=2)
                                for fc in range(fcn)]
                        for fc in range(fcn):
                            nc.tensor.matmul(
                                aggs[fc][:],
                                lhsT=slab_in[:, t, fc * 128:(fc + 1) * 128],
                                rhs=selfoh_sb[:, t, :], start=True, stop=False)
                        for i, (blk, p) in enumerate(blocks):
                            sgt = seg(blk // SEG)
                            col = blk % SEG
                            F = elem2 // 2
                            for fc in range(fcn):
                                nc.tensor.matmul(
                                    aggs[fc][:],
                                    lhsT=sgt[:, col,
                                             p * F + fc * 128:
                                             p * F + (fc + 1) * 128],
                                    rhs=oh_sb[:, blk * 128:(blk + 1) * 128],
                                    start=False, stop=(i == len(blocks) - 1))
                        agg_sbs = []
                        for fc in range(fcn):
                            a = wp.tile([128, 128], dt.bfloat16, tag=f"asb{fc}",
                                        name=f"asb{fc}")
                            if fc % 2 == 0:
                                nc.scalar.activation(a[:], aggs[fc][:], AF.Copy)
                            else:
                                nc.vector.tensor_copy(a[:], aggs[fc][:])
                            agg_sbs.append(a)
                        h_ps = psl.tile([128, fout], dt.float32, tag="hps",
                                        bufs=2)
                        nc.tensor.matmul(h_ps[:], lhsT=ones_bf[:], rhs=b_sb[:],
                                         start=True, stop=False)
                        for fc in range(fcn):
                            nc.tensor.matmul(h_ps[:], lhsT=agg_sbs[fc][:],
                                             rhs=w_rhs(fc), start=False,
                                             stop=(fc == fcn - 1))
                        if slab_out is not None:
                            nc.scalar.activation(slab_out[:, t, :], h_ps[:],
                                                 AF.Relu)
                            nc.sync.dma_start(own_out[t * 128:t * 128 + rows, :],
                                              slab_out[:rows, t, :])
                        else:
                            h3_sb = wp.tile([128, fout], dt.float8e4,
                                            tag="h3sb")
                            nc.scalar.activation(h3_sb[:], h_ps[:], AF.Relu)
                            nc.tensor.matmul(pool_ps[:], lhsT=ohb_sb[:, t, :],
                                             rhs=h3_sb[:], start=(t == 0),
                                             stop=(t == TPC - 1))

            do_layer(h1_all[:, :].rearrange("(n two) f -> n (two f)", two=2),
                     2 * H1, 1, lambda fc: w2_sb[:], b2_sb, h1sl, h2sl,
                     h2_own, None, "gsem2")
            ag_chunked(h2_own, h2_all, SH, H2)

            pool_ps = pps.tile([128, H3], dt.float32)
            do_layer(h2_all[:, :].rearrange("(n two) f -> n (two f)", two=2),
                     2 * H2, 2, lambda fc: w3_sb[:, fc, :], b3_sb, h2sl, None,
                     None, pool_ps, "gsem3")

            pool_sb = wp.tile([128, H3], dt.float32, tag="poolsb")
            nc.vector.tensor_scalar(pool_sb[:], pool_ps[:], cinv_sb[:, :1],
                                    None, OP.mult)
            pool_bf = wp.tile([128, H3], dt.bfloat16, tag="poolbf")
            nc.scalar.activation(pool_bf[:], pool_sb[:], AF.Copy)
            nc.sync.dma_start(pool_own[:], pool_bf[:])
            ag(pool_own[:, :], pool_all)

            # ---------------- FC head (replicated) --------------------------
            with tc.tile_pool(name="psf", bufs=2, space="PSUM") as psf:
                poolT = []
                for fc in range(4):
                    pt = cp.tile([128, G], dt.float32, tag=f"poolT{fc}")
                    nc.gpsimd.memset(pt[:], 0.0)
                    poolT.append(pt)
                for c in range(NC):
                    pc_sb = wp.tile([128, H3], dt.bfloat16, tag="pc", bufs=4)
                    nc.sync.dma_start(pc_sb[:], pool_all[c * 128:(c + 1) * 128, :])
                    wcols = min(128, G - g0s[c])
                    for fc in range(4):
                        tp_ps = psf.tile([128, 128], dt.bfloat16, tag="tp",
                                         bufs=4)
                        nc.tensor.transpose(tp_ps[:],
                                            pc_sb[:, fc * 128:(fc + 1) * 128],
                                            ident_bf[:])
                        sl = poolT[fc][:, g0s[c]:g0s[c] + wcols]
                        nc.vector.tensor_tensor(sl, sl, tp_ps[:, :wcols], OP.add)
                for gh in range(2):
                    fc_ps = psf.tile([128, 1], dt.float32, tag="fcps")
                    nc.tensor.matmul(fc_ps[:], lhsT=ones_f32[:], rhs=bfc_sb[:],
                                     start=True, stop=False)
                    for fc in range(4):
                        nc.tensor.matmul(fc_ps[:],
                                         lhsT=poolT[fc][:, gh * 128:(gh + 1) * 128],
                                         rhs=wfc_sb[:, fc:fc + 1],
                                         start=False, stop=(fc == 3))
                    o_sb = wp.tile([128, 1], dt.float32, tag="osb")
                    nc.scalar.activation(o_sb[:], fc_ps[:], AF.Sigmoid)
                    nc.sync.dma_start(out_ext[gh * 128:(gh + 1) * 128, :], o_sb[:])

    nc.compile()
    return nc


def _install_profile_hook():
    import importlib
    try:
        importlib.import_module("antenv.axon_hooks")
        return
    except ImportError:
        pass
    import types
    import ctypes
    import contextlib
    so_path = "/opt/axon/libaxon_pjrt.so"
    mod = types.ModuleType("antenv.axon_hooks")
    _state = {"hook": None}

    def set_axon_ntff_profile_hook(h):
        _state["hook"] = h

    def get_axon_ntff_profile_hook():
        if _state["hook"] is None and os.path.exists(so_path):
            lib = ctypes.CDLL(so_path)
            if hasattr(lib, "axon_start_nrt_profile"):
                lib.axon_start_nrt_profile.argtypes = [
                    ctypes.POINTER(ctypes.c_int64), ctypes.c_size_t]
                lib.axon_start_nrt_profile.restype = ctypes.c_int64
                lib.axon_stop_nrt_profile.argtypes = [ctypes.c_char_p]
                lib.axon_stop_nrt_profile.restype = ctypes.c_int64

                @contextlib.contextmanager
                def _hook(output_dir, device_ids):
                    import jax
                    jax.devices()
                    if device_ids:
                        ids = (ctypes.c_int64 * len(device_ids))(*device_ids)
                        rc = lib.axon_start_nrt_profile(ids, len(device_ids))
                    else:
                        rc = lib.axon_start_nrt_profile(None, 0)
                    if rc != 0:
                        raise RuntimeError(f"axon_start_nrt_profile rc={rc}")
                    try:
                        yield
                    finally:
                        n = lib.axon_stop_nrt_profile(str(output_dir).encode())
                        print(f"profile: {n} file(s) written to {output_dir}")

                _state["hook"] = _hook
        return _state["hook"]

    mod.set_axon_ntff_profile_hook = set_axon_ntff_profile_hook
    mod.get_axon_ntff_profile_hook = get_axon_ntff_profile_hook
    sys.modules["antenv.axon_hooks"] = mod


def kernel(**inputs):
    global LAST_EXEC_NS
    from concourse.bass_utils import run_bass_kernel_spmd

    per_core, struct = _prep(inputs["x"], inputs["edge_index"],
                             inputs["edge_weight"], inputs["batch"])

    key = (struct["CH"], tuple(map(tuple, struct["ch_uni"])),
           tuple(struct["g0s"]))
    if key not in _CACHE:
        _CACHE[key] = _build(struct)
    nc = _CACHE[key]

    W1 = np.asarray(inputs["W1"], np.float32)
    W2 = np.asarray(inputs["W2"], np.float32)
    W3 = np.asarray(inputs["W3"], np.float32)
    Wfc = np.asarray(inputs["Wfc"], np.float32)
    shared = dict(
        w1=W1.astype(bf16),
        w2=W2.astype(bf16),
        w3=np.ascontiguousarray(
            W3.reshape(2, 128, H3).transpose(1, 0, 2)).astype(bf16),
        wfc=np.ascontiguousarray(Wfc.reshape(4, 128).T).astype(np.float32),
        b1=np.asarray(inputs["b1"], np.float32).reshape(1, H1).astype(bf16),
        b2=np.asarray(inputs["b2"], np.float32).reshape(1, H2).astype(bf16),
        b3=np.asarray(inputs["b3"], np.float32).reshape(1, H3).astype(bf16),
        bfc=np.asarray(inputs["bfc"], np.float32).reshape(1, 1),
    )
    in_maps = [{**shared, **pc} for pc in per_core]

    if TRACE:
        _install_profile_hook()
    res = run_bass_kernel_spmd(nc, in_maps, list(range(NC)), trace=TRACE)
    LAST_EXEC_NS = res.exec_time_ns
    return res.results[0]["out"]



# revision 41
# speedup vs baseline: 1.1571x; 1.0442x over previous
"""GCN (3-layer + global mean pool + FC/sigmoid) on 8 Trainium2 NeuronCores, v2.

Aggregate-first node-sharded design. One canonical per-core edge order —
sorted by (dst tile, src parity, src) — is shared by all three layers: the
same fp8 one-hot scatter matrices drive layer 1/2/3 chunk matmuls, and
self-loops use per-tile bf16 diagonal matmuls from SBUF slabs.

Layer 1 needs no device gather at all: the host pre-gathers x[src] into the
canonical stream (xg, fp8). Layers 2/3 gather h1/h2 rows from fp8
"pair" tables ([N/2, 2F], one pair per 256B+ row, int16 pair indices) that
are exchanged with a single AllGather each. Tables, one-hot norms, and the
gathered message blocks are all fp8 (validated ~1e-3 max-rel); slabs,
weights, and the pooling path stay bf16/f32.
"""
import sys
import os

for _p in ("/opt/trn_rl_repo", "/root/.axon_site/_ro/trn_rl_repo"):
    if os.path.isdir(_p) and _p not in sys.path:
        sys.path.append(_p)

import numpy as np
import ml_dtypes

bf16 = ml_dtypes.bfloat16
f8 = ml_dtypes.float8_e4m3

N = 50000
E = 150000
G = 256
NC = 8
SH = N // NC             # 6250 nodes per core
TPC = (SH + 127) // 128  # 49 tiles per core
H1, H2, H3 = 128, 256, 512
SEG = 8                   # chunks per gather (1024 descs = single-packet cap)
GBUFS = int(os.environ.get('KGBUFS', '20'))

TRACE = False
LAST_EXEC_NS = None
_CACHE = {}
AGCHUNK = int(os.environ.get('KAGCHUNK', '1'))


def _prep(x, edge_index, edge_weight, batch):
    """Host-side graph preprocessing -> per-core input arrays + structure."""
    x = np.asarray(x, np.float32)
    ei = np.asarray(edge_index)
    ew = np.asarray(edge_weight, np.float32)
    batch = np.asarray(batch).astype(np.int64)

    src = ei[0].astype(np.int64)
    dst = ei[1].astype(np.int64)
    deg = np.bincount(dst, weights=ew, minlength=N).astype(np.float32) + 1.0
    dinv = (1.0 / np.sqrt(deg)).astype(np.float32)
    norm = (dinv[src] * ew * dinv[dst]).astype(np.float32)
    norm_self = (dinv * dinv).astype(np.float32)

    core = dst // SH
    tile_l = (dst % SH) // 128
    par = src % 2
    key = (core * TPC + tile_l) * 2 + par
    order = np.lexsort((src, key))
    src_s, dst_s, norm_s, key_s = src[order], dst[order], norm[order], key[order]

    cnt = np.bincount(key, minlength=NC * TPC * 2).reshape(NC, TPC, 2)
    ch_uni = np.ceil(cnt / 128).astype(np.int64).max(axis=0)   # [TPC, 2]
    CH = int(ch_uni.sum())
    # global block index of (tile, parity, k): blocks laid out tile-major
    blk_base = np.zeros((TPC, 2), np.int64)
    running = 0
    for t in range(TPC):
        for p in range(2):
            blk_base[t, p] = running
            running += ch_uni[t, p]

    block_start = np.zeros(NC * TPC * 2 + 1, np.int64)
    block_start[1:] = np.cumsum(cnt.reshape(-1))

    cntg = np.bincount(batch, minlength=G).astype(np.float32)
    cntinv_g = (1.0 / np.maximum(cntg, 1.0)).astype(np.float32)

    g0s = [int(batch[c * SH]) for c in range(NC)]
    for c in range(NC):
        assert int(batch[(c + 1) * SH - 1]) - g0s[c] < 128, "graph window > 128"

    def idx_pack(lin):
        a = lin.reshape(-1, 16).T
        return np.ascontiguousarray(np.tile(a, (8, 1)))

    per_core = []
    for c in range(NC):
        idx = np.zeros(CH * 128, np.int16)          # pair indices
        dstl = np.full(CH * 128, -1.0, np.float32)  # dst within tile, -1 = pad
        normv = np.zeros(CH * 128, np.float32)
        xg = np.zeros((CH * 128, 8), np.float32)
        for t in range(TPC):
            for p in range(2):
                bkey = (c * TPC + t) * 2 + p
                b0, b1 = block_start[bkey], block_start[bkey + 1]
                n = b1 - b0
                s0 = int(blk_base[t, p]) * 128
                idx[s0:s0 + n] = (src_s[b0:b1] // 2).astype(np.int16)
                dstl[s0:s0 + n] = (dst_s[b0:b1] - (c * SH + t * 128)).astype(
                    np.float32)
                normv[s0:s0 + n] = norm_s[b0:b1]
                xg[s0:s0 + n] = x[src_s[b0:b1]]

        # one-hot scatter matrices [128, CH*128] fp8: oh[p, blk*128 + d] =
        # norm of message (blk, p) if its dst-local == d
        nq = normv.astype(f8).astype(np.float32)
        ohm = np.zeros((CH * 128, 128), np.float32)
        valid = dstl >= 0
        ohm[np.arange(CH * 128)[valid], dstl[valid].astype(np.int64)] = nq[valid]
        ohm = ohm.reshape(CH, 128, 128).transpose(1, 0, 2).reshape(128, CH * 128)

        ns_pad = np.zeros(TPC * 128, np.float32)
        ns_pad[:SH] = norm_self[c * SH:(c + 1) * SH]
        selfoh = np.zeros((128, TPC, 128), np.float32)
        pr = np.arange(128)
        for t in range(TPC):
            selfoh[pr, t, pr] = ns_pad[t * 128:(t + 1) * 128]

        xsl = np.zeros((TPC * 128, 8), np.float32)
        xsl[:SH] = x[c * SH:(c + 1) * SH]

        bl = np.full((TPC * 128,), -1, np.int64)
        bl[:SH] = batch[c * SH:(c + 1) * SH] - g0s[c]
        ohb = np.zeros((TPC * 128, 128), np.float32)
        vb = bl >= 0
        ohb[np.arange(TPC * 128)[vb], bl[vb]] = 1.0
        ohb = ohb.reshape(TPC, 128, 128).transpose(1, 0, 2)
        # (loaded as fp8 below; exact for 0/1 values)

        ig = g0s[c] + np.arange(128)
        cinv = np.where(ig < G, cntinv_g[np.minimum(ig, G - 1)], 0.0)

        per_core.append(dict(
            idx=idx_pack(idx),
            oh=np.ascontiguousarray(ohm).astype(f8),
            selfoh=np.ascontiguousarray(selfoh.reshape(128, TPC * 128)).astype(f8),
            xg=np.ascontiguousarray(
                xg.reshape(CH, 128, 8).transpose(1, 0, 2)).astype(f8),
            xsl=np.ascontiguousarray(
                xsl.reshape(TPC, 128, 8).transpose(1, 0, 2)).astype(f8),
            ohb=np.ascontiguousarray(ohb.reshape(128, TPC * 128)).astype(f8),
            cntinv=cinv.astype(np.float32).reshape(128, 1),
        ))

    struct = dict(CH=CH,
                  ch_uni=[[int(ch_uni[t, p]) for p in range(2)]
                          for t in range(TPC)],
                  blk_base=[[int(blk_base[t, p]) for p in range(2)]
                            for t in range(TPC)],
                  g0s=g0s)
    return per_core, struct


def _build(struct):
    import concourse.bacc as bacc
    import concourse.mybir as mybir
    import concourse.tile as tile
    from concourse.masks import make_identity

    dt = mybir.dt
    AF = mybir.ActivationFunctionType
    OP = mybir.AluOpType

    CH = struct["CH"]
    ch_uni = struct["ch_uni"]
    blk_base = struct["blk_base"]
    g0s = struct["g0s"]
    NSEG = (CH + SEG - 1) // SEG

    nc = bacc.Bacc("TRN2", target_bir_lowering=False, debug=False,
                   num_devices=NC, num_swdge_queues=4)

    w1_in = nc.dram_tensor("w1", [8, H1], dt.bfloat16, kind="ExternalInput")
    w2_in = nc.dram_tensor("w2", [H1, H2], dt.bfloat16, kind="ExternalInput")
    w3_in = nc.dram_tensor("w3", [128, 2, H3], dt.bfloat16, kind="ExternalInput")
    wfc_in = nc.dram_tensor("wfc", [128, 4], dt.float32, kind="ExternalInput")
    b1_in = nc.dram_tensor("b1", [1, H1], dt.bfloat16, kind="ExternalInput")
    b2_in = nc.dram_tensor("b2", [1, H2], dt.bfloat16, kind="ExternalInput")
    b3_in = nc.dram_tensor("b3", [1, H3], dt.bfloat16, kind="ExternalInput")
    bfc_in = nc.dram_tensor("bfc", [1, 1], dt.float32, kind="ExternalInput")

    idx_in = nc.dram_tensor("idx", [128, CH * 8], dt.int16, kind="ExternalInput")
    oh_in = nc.dram_tensor("oh", [128, CH * 128], dt.float8e4,
                           kind="ExternalInput")
    selfoh_in = nc.dram_tensor("selfoh", [128, TPC * 128], dt.float8e4,
                               kind="ExternalInput")
    xg_in = nc.dram_tensor("xg", [128, CH * 8], dt.float8e4,
                           kind="ExternalInput")
    xsl_in = nc.dram_tensor("xsl", [128, TPC * 8], dt.float8e4,
                            kind="ExternalInput")
    ohb_in = nc.dram_tensor("ohb", [128, TPC * 128], dt.float8e4,
                            kind="ExternalInput")
    cinv_in = nc.dram_tensor("cntinv", [128, 1], dt.float32, kind="ExternalInput")
    out_ext = nc.dram_tensor("out", [G, 1], dt.float32, kind="ExternalOutput")

    with tile.TileContext(nc) as tc:
        with tc.tile_pool(name="const", bufs=1) as cp, \
             tc.tile_pool(name="meta", bufs=1) as mp, \
             tc.tile_pool(name="gseg", bufs=GBUFS) as gp, \
             tc.tile_pool(name="work", bufs=3) as wp, \
             tc.tile_pool(name="slab", bufs=1) as slp, \
             tc.tile_pool(name="pps", bufs=1, space="PSUM") as pps, \
             tc.tile_pool(name="dram", bufs=1, space="DRAM") as dram:

            def load(pool, t_in, shape, dtype, tag):
                t = pool.tile(shape, dtype, tag=tag)
                nc.sync.dma_start(t[:], t_in[:])
                return t

            w1_sb = load(cp, w1_in, [8, H1], dt.bfloat16, "w1")
            w2_sb = load(cp, w2_in, [H1, H2], dt.bfloat16, "w2")
            w3_sb = load(cp, w3_in, [128, 2, H3], dt.bfloat16, "w3")
            wfc_sb = load(cp, wfc_in, [128, 4], dt.float32, "wfc")
            b1_sb = load(cp, b1_in, [1, H1], dt.bfloat16, "b1")
            b2_sb = load(cp, b2_in, [1, H2], dt.bfloat16, "b2")
            b3_sb = load(cp, b3_in, [1, H3], dt.bfloat16, "b3")
            bfc_sb = load(cp, bfc_in, [1, 1], dt.float32, "bfc")
            idx_sb = load(mp, idx_in, [128, CH * 8], dt.int16, "idx")
            # piecewise loads: tile 0's matmuls gate only on the first slice
            oh_sb = mp.tile([128, CH * 128], dt.float8e4, tag="oh")
            ohq = (CH * 128) // 4
            for _q in range(4):
                _e = CH * 128 if _q == 3 else (_q + 1) * ohq
                nc.sync.dma_start(oh_sb[:, _q * ohq:_e],
                                  oh_in[:, _q * ohq:_e])
            selfoh_sb = mp.tile([128, TPC, 128], dt.float8e4, tag="selfoh")
            soq = TPC // 4
            for _q in range(4):
                _e = TPC if _q == 3 else (_q + 1) * soq
                nc.sync.dma_start(
                    selfoh_sb[:, _q * soq:_e, :],
                    selfoh_in[:].rearrange("p (t f) -> p t f", f=128)
                    [:, _q * soq:_e, :])
            xg_sb = load(mp, xg_in, [128, CH, 8], dt.float8e4, "xg")
            xsl_sb = load(mp, xsl_in, [128, TPC, 8], dt.float8e4, "xsl")
            ohb_sb = load(mp, ohb_in, [128, TPC, 128], dt.float8e4, "ohb")
            cinv_sb = load(mp, cinv_in, [128, 1], dt.float32, "cinv")

            ones_bf = cp.tile([1, 128], dt.bfloat16, tag="ones_bf")
            nc.gpsimd.memset(ones_bf[:], 1.0)
            ones_f32 = cp.tile([1, 128], dt.float32, tag="ones_f32")
            nc.gpsimd.memset(ones_f32[:], 1.0)
            ident = cp.tile([128, 128], dt.float32, tag="ident")
            make_identity(nc, ident[:])
            ident_bf = cp.tile([128, 128], dt.bfloat16, tag="identbf")
            nc.vector.tensor_copy(ident_bf[:], ident[:])

            h1sl = slp.tile([128, TPC, H1], dt.float8e4, tag="h1sl")
            h2sl = slp.tile([128, TPC, H2], dt.float8e4, tag="h2sl")

            pool_dmas = []  # 1-elem views of each pool-DMA output, lane order
            sink_sb = cp.tile([1, 4], dt.float8e4, tag="sink")

            h1_own = dram.tile([SH, H1], dt.float8e4, tag="h1own")
            h1_all = dram.tile([N, H1], dt.float8e4, tag="h1all",
                               addr_space="Shared")
            h2_own = dram.tile([SH, H2], dt.float8e4, tag="h2own")
            h2_all = dram.tile([N, H2], dt.float8e4, tag="h2all",
                               addr_space="Shared")
            pool_own = dram.tile([128, H3], dt.bfloat16, tag="plown")
            pool_all = dram.tile([NC * 128, H3], dt.bfloat16, tag="plall",
                                 addr_space="Shared")

            def tile_blocks(t):
                """[(blk, par), ...] for tile t in stream order."""
                out = []
                for p in range(2):
                    for k in range(ch_uni[t][p]):
                        out.append((blk_base[t][p] + k, p))
                return out

            def ag(in_ap, out_t):
                nc.gpsimd.collective_compute(
                    "AllGather", mybir.AluOpType.bypass,
                    replica_groups=[list(range(NC))],
                    ins=[in_ap.opt() if hasattr(in_ap, 'opt') else in_ap],
                    outs=[out_t.opt()])

            def ag_chunked(own_t, all_t, sh, f):
                """AllGather own [sh, f] -> all [NC*sh, f] in AGCHUNK pieces
                so early pieces overlap the producing layer's tail."""
                if AGCHUNK <= 1:
                    ag(own_t[:, :], all_t)
                    return
                view = all_t[:].rearrange("(c r) f -> c r f", c=NC)
                bnd = [0]
                step = ((sh // AGCHUNK) // 128) * 128
                for k in range(1, AGCHUNK):
                    bnd.append(step * k)
                bnd.append(sh)
                for k in range(AGCHUNK):
                    r0, r1 = bnd[k], bnd[k + 1]
                    nc.gpsimd.collective_compute(
                        "AllGather", mybir.AluOpType.bypass,
                        replica_groups=[list(range(NC))],
                        ins=[own_t[r0:r1, :]],
                        outs=[view[:, r0:r1, :]])

            # ---------------- Layer 1 (host-gathered stream) ----------------
            with tc.tile_pool(name="ps1", bufs=1, space="PSUM") as ps1:
                for t in range(TPC):
                    rows = min(128, SH - t * 128)
                    blocks = tile_blocks(t)
                    aggx_ps = ps1.tile([8, 128], dt.float32, tag="aggx", bufs=2)
                    nc.tensor.matmul(aggx_ps[:], lhsT=xsl_sb[:, t, :],
                                     rhs=selfoh_sb[:, t, :],
                                     start=True, stop=False)
                    for i, (blk, p) in enumerate(blocks):
                        nc.tensor.matmul(aggx_ps[:], lhsT=xg_sb[:, blk, :],
                                         rhs=oh_sb[:, blk * 128:(blk + 1) * 128],
                                         start=False, stop=(i == len(blocks) - 1))
                    aggx_sb = wp.tile([8, 128], dt.bfloat16, tag="aggxsb",
                                      bufs=2)
                    nc.scalar.activation(aggx_sb[:], aggx_ps[:], AF.Copy)
                    h1_ps = ps1.tile([128, H1], dt.float32, tag="h1ps", bufs=2)
                    nc.tensor.matmul(h1_ps[:], lhsT=ones_bf[:], rhs=b1_sb[:],
                                     start=True, stop=False)
                    nc.tensor.matmul(h1_ps[:], lhsT=aggx_sb[:], rhs=w1_sb[:],
                                     start=False, stop=True)
                    # relu -> fp8 slab; DMA the table row straight from it
                    nc.scalar.activation(h1sl[:, t, :], h1_ps[:], AF.Relu)
                    nc.sync.dma_start(h1_own[t * 128:t * 128 + rows, :],
                                      h1sl[:rows, t, :])

            ag_chunked(h1_own, h1_all, SH, H1)

            # ---------------- Layers 2/3 (gather + scatter matmuls) --------
            def do_layer(tab_pairs, elem2, fcn, w_rhs, b_sb, slab_in, slab_out,
                         own_out, pool_ps, sem_name):
                # A gpsimd instruction carrying a semaphore wait serializes
                # the Q7 dispatch pipeline until it retires; waitless ones
                # pipeline freely (4-way desc-gen across SWDGE queues). So
                # keep the gathers waitless: a dummy 1-row table read absorbs
                # the collective-done wait, and before each gather whose
                # DMASW lane is being recycled (every 8th pool-DMA) a 1-elem
                # gpsimd copy reads the lane predecessor's output so Tile
                # attaches the recycle wait to the copy and elides it on the
                # gather itself.
                scrap = wp.tile([128, elem2], dt.float8e4, tag="scrap")
                nc.gpsimd.dma_start(scrap[:1, :], tab_pairs[0:1, :])
                pool_dmas.append(scrap[:1, 0:1])
                seg_tiles = []
                for s in range(NSEG):
                    k = len(pool_dmas)
                    if s % 4 == 0:
                        # absorb the recycle waits of the next wave of (up
                        # to) 4 gathers in one drain: after it, the wave
                        # dispatches waitless and desc-gens 4-way parallel
                        nwave = min(4, NSEG - s)
                        for j in range(nwave):
                            if k + j >= 8:
                                nc.gpsimd.tensor_copy(sink_sb[:1, 0:1],
                                                      pool_dmas[k + j - 8])
                    # one shared slot pool (sized for L3's 512B rows); L2's
                    # 256B rows use the contiguous first part of each slot
                    nch = min(SEG, CH - s * SEG)
                    slot = gp.tile([128, SEG * 2 * H2], dt.float8e4,
                                   tag="gseg")
                    t_ = slot[:, :SEG * elem2].rearrange(
                        "p (c e) -> p c e", e=elem2)
                    nc.gpsimd.dma_gather(
                        t_[:, :nch, :], tab_pairs,
                        idx_sb[:, s * SEG * 8:(s * SEG + nch) * 8],
                        nch * 128, nch * 128, elem2, queue_num=(s % 4))
                    pool_dmas.append(t_[:1, 0, 0:1])
                    seg_tiles.append(t_)

                def seg(s):
                    return seg_tiles[s]

                fout = H2 if fcn == 1 else H3
                with tc.tile_pool(name=f"psl{fcn}", bufs=1, space="PSUM") as psl:
                    for t in range(TPC):
                        rows = min(128, SH - t * 128)
                        blocks = tile_blocks(t)
                        aggs = [psl.tile([128, 128], dt.float32, tag=f"agg{fc}",
                                         name=f"agg{fc}", bufs=2)
                                for fc in range(fcn)]
                        for fc in range(fcn):
                            nc.tensor.matmul(
                                aggs[fc][:],
                                lhsT=slab_in[:, t, fc * 128:(fc + 1) * 128],
                                rhs=selfoh_sb[:, t, :], start=True, stop=False)
                        for i, (blk, p) in enumerate(blocks):
                            sgt = seg(blk // SEG)
                            col = blk % SEG
                            F = elem2 // 2
                            for fc in range(fcn):
                                nc.tensor.matmul(
                                    aggs[fc][:],
                                    lhsT=sgt[:, col,
                                             p * F + fc * 128:
                                             p * F + (fc + 1) * 128],
                                    rhs=oh_sb[:, blk * 128:(blk + 1) * 128],
                                    start=False, stop=(i == len(blocks) - 1))
                        agg_sbs = []
                        for fc in range(fcn):
                            a = wp.tile([128, 128], dt.bfloat16, tag=f"asb{fc}",
                                        name=f"asb{fc}")
                            if fc % 2 == 0:
                                nc.scalar.activation(a[:], aggs[fc][:], AF.Copy)
                            else:
                                nc.vector.tensor_copy(a[:], aggs[fc][:])
                            agg_sbs.append(a)
                        h_ps = psl.tile([128, fout], dt.float32, tag="hps",
                                        bufs=2)
                        nc.tensor.matmul(h_ps[:], lhsT=ones_bf[:], rhs=b_sb[:],
                                         start=True, stop=False)
                        for fc in range(fcn):
                            nc.tensor.matmul(h_ps[:], lhsT=agg_sbs[fc][:],
                                             rhs=w_rhs(fc), start=False,
                                             stop=(fc == fcn - 1))
                        if slab_out is not None:
                            nc.scalar.activation(slab_out[:, t, :], h_ps[:],
                                                 AF.Relu)
                            nc.sync.dma_start(own_out[t * 128:t * 128 + rows, :],
                                              slab_out[:rows, t, :])
                        else:
                            h3_sb = wp.tile([128, fout], dt.float8e4,
                                            tag="h3sb")
                            nc.scalar.activation(h3_sb[:], h_ps[:], AF.Relu)
                            nc.tensor.matmul(pool_ps[:], lhsT=ohb_sb[:, t, :],
                                             rhs=h3_sb[:], start=(t == 0),
                                             stop=(t == TPC - 1))

            do_layer(h1_all[:, :].rearrange("(n two) f -> n (two f)", two=2),
                     2 * H1, 1, lambda fc: w2_sb[:], b2_sb, h1sl, h2sl,
                     h2_own, None, "gsem2")
            ag_chunked(h2_own, h2_all, SH, H2)

            pool_ps = pps.tile([128, H3], dt.float32)
            do_layer(h2_all[:, :].rearrange("(n two) f -> n (two f)", two=2),
                     2 * H2, 2, lambda fc: w3_sb[:, fc, :], b3_sb, h2sl, None,
                     None, pool_ps, "gsem3")

            pool_sb = wp.tile([128, H3], dt.float32, tag="poolsb")
            nc.vector.tensor_scalar(pool_sb[:], pool_ps[:], cinv_sb[:, :1],
                                    None, OP.mult)
            pool_bf = wp.tile([128, H3], dt.bfloat16, tag="poolbf")
            nc.scalar.activation(pool_bf[:], pool_sb[:], AF.Copy)
            nc.sync.dma_start(pool_own[:], pool_bf[:])
            ag(pool_own[:, :], pool_all)

            # ---------------- FC head (replicated) --------------------------
            with tc.tile_pool(name="psf", bufs=2, space="PSUM") as psf:
                poolT = []
                for fc in range(4):
                    pt = cp.tile([128, G], dt.float32, tag=f"poolT{fc}")
                    nc.gpsimd.memset(pt[:], 0.0)
                    poolT.append(pt)
                for c in range(NC):
                    pc_sb = wp.tile([128, H3], dt.bfloat16, tag="pc", bufs=4)
                    nc.sync.dma_start(pc_sb[:], pool_all[c * 128:(c + 1) * 128, :])
                    wcols = min(128, G - g0s[c])
                    for fc in range(4):
                        tp_ps = psf.tile([128, 128], dt.bfloat16, tag="tp",
                                         bufs=4)
                        nc.tensor.transpose(tp_ps[:],
                                            pc_sb[:, fc * 128:(fc + 1) * 128],
                                            ident_bf[:])
                        sl = poolT[fc][:, g0s[c]:g0s[c] + wcols]
                        nc.vector.tensor_tensor(sl, sl, tp_ps[:, :wcols], OP.add)
                for gh in range(2):
                    fc_ps = psf.tile([128, 1], dt.float32, tag="fcps")
                    nc.tensor.matmul(fc_ps[:], lhsT=ones_f32[:], rhs=bfc_sb[:],
                                     start=True, stop=False)
                    for fc in range(4):
                        nc.tensor.matmul(fc_ps[:],
                                         lhsT=poolT[fc][:, gh * 128:(gh + 1) * 128],
                                         rhs=wfc_sb[:, fc:fc + 1],
                                         start=False, stop=(fc == 3))
                    o_sb = wp.tile([128, 1], dt.float32, tag="osb")
                    nc.scalar.activation(o_sb[:], fc_ps[:], AF.Sigmoid)
                    nc.sync.dma_start(out_ext[gh * 128:(gh + 1) * 128, :], o_sb[:])

    nc.compile()
    return nc


def _install_profile_hook():
    import importlib
    try:
        importlib.import_module("antenv.axon_hooks")
        return
    except ImportError:
        pass
    import types
    import ctypes
    import contextlib
    so_path = "/opt/axon/libaxon_pjrt.so"
    mod = types.ModuleType("antenv.axon_hooks")
    _state = {"hook": None}

    def set_axon_ntff_profile_hook(h):
        _state["hook"] = h

    def get_axon_ntff_profile_hook():
        if _state["hook"] is None and os.path.exists(so_path):
            lib = ctypes.CDLL(so_path)
            if hasattr(lib, "axon_start_nrt_profile"):
                lib.axon_start_nrt_profile.argtypes = [
                    ctypes.POINTER(ctypes.c_int64), ctypes.c_size_t]
                lib.axon_start_nrt_profile.restype = ctypes.c_int64
                lib.axon_stop_nrt_profile.argtypes = [ctypes.c_char_p]
                lib.axon_stop_nrt_profile.restype = ctypes.c_int64

                @contextlib.contextmanager
                def _hook(output_dir, device_ids):
                    import jax
                    jax.devices()
                    if device_ids:
                        ids = (ctypes.c_int64 * len(device_ids))(*device_ids)
                        rc = lib.axon_start_nrt_profile(ids, len(device_ids))
                    else:
                        rc = lib.axon_start_nrt_profile(None, 0)
                    if rc != 0:
                        raise RuntimeError(f"axon_start_nrt_profile rc={rc}")
                    try:
                        yield
                    finally:
                        n = lib.axon_stop_nrt_profile(str(output_dir).encode())
                        print(f"profile: {n} file(s) written to {output_dir}")

                _state["hook"] = _hook
        return _state["hook"]

    mod.set_axon_ntff_profile_hook = set_axon_ntff_profile_hook
    mod.get_axon_ntff_profile_hook = get_axon_ntff_profile_hook
    sys.modules["antenv.axon_hooks"] = mod


def kernel(**inputs):
    global LAST_EXEC_NS
    from concourse.bass_utils import run_bass_kernel_spmd

    per_core, struct = _prep(inputs["x"], inputs["edge_index"],
                             inputs["edge_weight"], inputs["batch"])

    key = (struct["CH"], tuple(map(tuple, struct["ch_uni"])),
           tuple(struct["g0s"]))
    if key not in _CACHE:
        _CACHE[key] = _build(struct)
    nc = _CACHE[key]

    W1 = np.asarray(inputs["W1"], np.float32)
    W2 = np.asarray(inputs["W2"], np.float32)
    W3 = np.asarray(inputs["W3"], np.float32)
    Wfc = np.asarray(inputs["Wfc"], np.float32)
    shared = dict(
        w1=W1.astype(bf16),
        w2=W2.astype(bf16),
        w3=np.ascontiguousarray(
            W3.reshape(2, 128, H3).transpose(1, 0, 2)).astype(bf16),
        wfc=np.ascontiguousarray(Wfc.reshape(4, 128).T).astype(np.float32),
        b1=np.asarray(inputs["b1"], np.float32).reshape(1, H1).astype(bf16),
        b2=np.asarray(inputs["b2"], np.float32).reshape(1, H2).astype(bf16),
        b3=np.asarray(inputs["b3"], np.float32).reshape(1, H3).astype(bf16),
        bfc=np.asarray(inputs["bfc"], np.float32).reshape(1, 1),
    )
    in_maps = [{**shared, **pc} for pc in per_core]

    if TRACE:
        _install_profile_hook()
    res = run_bass_kernel_spmd(nc, in_maps, list(range(NC)), trace=TRACE)
    LAST_EXEC_NS = res.exec_time_ns
    return res.results[0]["out"]

